# revision 46
# baseline (speedup 1.0000x reference)
"""Cross-attention block kernel for Trainium2 (Bass/Tile), SPMD over 8 cores.

Sharding: data-parallel over batch B=8 -> one batch element per NeuronCore.
Per core:
  xn  = LayerNorm(xt) * w + b                      [4096, 128]
  cn  = LayerNorm(context) * cw + cb               [256, 768]
  k,v = cn @ Wkv (+ null kv row), q = xn @ Wq
  sim = q k^T / 8, masked softmax over keys, out = attn v
  final = out @ Wout + bout + xn                   [4096, 128]

Design (from the TRN2 cost model + measured behavior):
  *  Every matmul uses an f32r STATIONARY: f32r self-loads, emitting no
     Ldweights instruction (bf16/fp8 stationaries cost ~357ns of PE
     sequencer each), and f32r moving data at free-size >= 256 runs at
     1 cycle/row, the same as bf16.
  *  KQ trick: sim_h^T = (Wq_h @ (k_h - k_null)^T)^T @ xn^T with K=128
     everywhere; q is never formed and the null key folds in via softmax
     shift invariance (its probability is exactly 1).
  *  Head-PAIR packing: AV numerators for heads (2p, 2p+1) accumulate
     into one PSUM tile po[0:64]/po[64:128] via zero-padded [128,128]
     stationaries, and the denominators are computed BY THE PE into a
     second tile pden with mask-column stationaries (each head's den
     replicated across 64 rows; +1 for the null key is a rank-1 seed).
     The softmax divide is then one reciprocal_approx_fast (DVE) and one
     fused (po + v_null) * rden scalar_tensor_tensor per (pair, chunk),
     all on dense [128, 512] tiles: no partition_broadcast, no
     single-partition ops, no per-element division.
  *  tc.For_i barriers every trip, so nothing overlaps across iterations.
     The per-iteration head is therefore latency-optimized: the context
     path (which gates kv -> KQ -> sims) is emitted first, and the xt
     LayerNorm runs as 8 groups of 4 chunks pipelined across DVE (stats)
     -> Pool (normalize) -> PE (transpose) -> Act (copy, folding *w+b as
     per-partition scale/bias on the PSUM->SBUF move).
  *  Attention runs as 32 software-pipelined (pair, chunk) units: sim+exp
     of unit k+1 issue before the AV/den/divide of unit k; the projection
     (+ residual via an identity-matmul PSUM seed + bias) trails one unit.
  *  The output stays [C, TOK] in DRAM (contiguous per-partition stores
     straight from the bias-add tile) and is untransposed on the host,
     saving 32 PE transposes + copies per iteration.
"""

import numpy as np

import concourse.bacc as bacc
import concourse.bass as bass
import concourse.mybir as mybir
import concourse.tile as tile
from concourse.bass_utils import run_bass_kernel_spmd
from concourse.masks import make_identity

B, XS, YS, C = 8, 64, 64, 128
CTX, N, H, D = 768, 256, 8, 64
HID = H * D          # 512
TOK = XS * YS        # 4096 tokens per batch element
TCH = 512            # tokens per chunk (PSUM bank free size in fp32)
NT = TOK // TCH      # 8 token chunks
NP = H // 2          # 4 head pairs
NCORES = 8
F32 = mybir.dt.float32
F32R = mybir.dt.float32r
BF16 = mybir.dt.bfloat16
EPS = 1e-5
SCALE = D ** -0.5
Exp = mybir.ActivationFunctionType.Exp
Sqrt = mybir.ActivationFunctionType.Sqrt
Ident = mybir.ActivationFunctionType.Identity
Copy = mybir.ActivationFunctionType.Copy
SUB = mybir.AluOpType.subtract
MUL = mybir.AluOpType.mult
ADD = mybir.AluOpType.add


def build(n_iters: int = 1):
    nc = bacc.Bacc("TRN2", target_bir_lowering=False, debug=False,
                   num_devices=NCORES)

    xt_d = nc.dram_tensor("xt", [TOK, C], F32, kind="ExternalInput")
    ctx_d = nc.dram_tensor("context", [N, CTX], F32, kind="ExternalInput")
    mask_d = nc.dram_tensor("mask", [N], mybir.dt.uint8, kind="ExternalInput")
    nw_d = nc.dram_tensor("norm_w", [C], F32, kind="ExternalInput")
    nb_d = nc.dram_tensor("norm_b", [C], F32, kind="ExternalInput")
    cw_d = nc.dram_tensor("ctx_norm_w", [CTX], F32, kind="ExternalInput")
    cb_d = nc.dram_tensor("ctx_norm_b", [CTX], F32, kind="ExternalInput")
    wq_d = nc.dram_tensor("Wq", [C, HID], F32, kind="ExternalInput")
    wkv_d = nc.dram_tensor("Wkv", [CTX, 2 * HID], F32, kind="ExternalInput")
    nkv_d = nc.dram_tensor("null_kv", [2, D], F32, kind="ExternalInput")
    wout_d = nc.dram_tensor("Wout", [HID, C], F32, kind="ExternalInput")
    bout_d = nc.dram_tensor("bout", [C], F32, kind="ExternalInput")
    out_d = nc.dram_tensor("out", [C, TOK], F32, kind="ExternalOutput")

    def one_part(handle, off, n_free):
        return bass.AP(handle, off, [[0, 1], [1, n_free]])

    def col_ap(handle, n_part):
        return bass.AP(handle, 0, [[1, n_part], [1, 1]])

    with tile.TileContext(nc) as tc:
        with (
            tc.tile_pool(name="const", bufs=1) as const,
            tc.tile_pool(name="wides", bufs=1) as wides,
            tc.tile_pool(name="work", bufs=1) as work,
            tc.tile_pool(name="iterw", bufs=2) as iterw,
            tc.tile_pool(name="small", bufs=2) as small,
            tc.tile_pool(name="pep", bufs=4) as pep,
            tc.tile_pool(name="op", bufs=2) as op_pool,
            tc.tile_pool(name="rp", bufs=2) as rp,
            tc.tile_pool(name="ftp", bufs=2) as ftp,
            tc.tile_pool(name="fop", bufs=2) as fop,
            tc.tile_pool(name="pa", bufs=2, space=bass.MemorySpace.PSUM) as pa,
            tc.tile_pool(name="pb", bufs=4, space=bass.MemorySpace.PSUM) as pb,
        ):
            # ======================= constants ========================
            ident = const.tile([128, 128], F32)
            make_identity(nc, ident)
            ident_r = const.tile([128, 128], F32R)
            nc.scalar.copy(out=ident_r, in_=ident)
            eps_t = const.tile([128, 1], F32)
            nc.vector.memset(eps_t, EPS)
            ones64 = const.tile([128, 64], F32)      # sden source
            nc.vector.memset(ones64, 1.0)
            onespad = const.tile([128, 4 * D], F32)
            nc.vector.memset(onespad, 1.0)
            ones_f = const.tile([1, TCH], F32)
            nc.vector.memset(ones_f, 1.0)
            ones_row = const.tile([1, TCH], F32R)    # seed moving / stats
            nc.scalar.copy(out=ones_row, in_=ones_f)

            # ============ loop-invariant weight loads + prep ==========
            # Wkv [128, 6, 1024] (cb-blocked rows), f32 (stationary: bitcast r)
            wkv_sb = wides.tile([128, 6, 2 * HID], F32R, tag="wkv")
            nc.gpsimd.dma_start(
                out=wkv_sb,
                in_=bass.AP(wkv_d, 0, [[2 * HID, 128], [128 * 2 * HID, 6],
                                       [1, 2 * HID]]))
            # Wout pair-packed: [128 (= hid within pair), pair, C]
            wout_pr = wides.tile([128, NP, C], F32R, tag="woutp")
            nc.gpsimd.dma_start(
                out=wout_pr,
                in_=wout_d.ap().rearrange("(a p) c -> p a c", p=128))
            # norm vectors as per-partition columns
            nw_col = wides.tile([C, 1], F32, tag="nw")
            nc.sync.dma_start(out=nw_col, in_=col_ap(nw_d, C))
            nb_col = wides.tile([C, 1], F32, tag="nb")
            nc.sync.dma_start(out=nb_col, in_=col_ap(nb_d, C))
            cw_cols = wides.tile([128, 6], F32, tag="cw")
            nc.sync.dma_start(out=cw_cols,
                              in_=bass.AP(cw_d, 0, [[1, 128], [128, 6]]))
            cb_cols = wides.tile([128, 6], F32, tag="cb")
            nc.sync.dma_start(out=cb_cols,
                              in_=bass.AP(cb_d, 0, [[1, 128], [128, 6]]))
            bout_col = wides.tile([C, 1], F32, tag="bout")
            nc.sync.dma_start(out=bout_col, in_=col_ap(bout_d, C))
            # k_null stacked twice on partitions (both heads of an hb pair)
            kn_col = wides.tile([128, 1], F32, tag="kn")
            nc.sync.dma_start(out=kn_col[0:D, :], in_=col_ap(nkv_d, D))
            nc.sync.dma_start(out=kn_col[D:128, :],
                              in_=bass.AP(nkv_d, 0, [[1, D], [1, 1]]))
            nkn_col = wides.tile([128, 1], F32, tag="nkn")
            nc.vector.tensor_scalar_mul(nkn_col, kn_col, -1.0)
            # v_null stacked twice on partitions: folded into the divide STT
            vn2_col = wides.tile([128, 1], F32, tag="vn2")
            nc.sync.dma_start(out=vn2_col[0:D, :],
                              in_=bass.AP(nkv_d, D, [[1, D], [1, 1]]))
            nc.sync.dma_start(out=vn2_col[D:128, :],
                              in_=bass.AP(nkv_d, D, [[1, D], [1, 1]]))
            onescol = wides.tile([1, 128], F32R, tag="ones1")
            nc.scalar.copy(out=onescol, in_=ones_f[:, 0:128])
            # wqTp[:, h, :]: rows (h%2)*64..+64 = Wq_h^T, other rows 0
            wq_sb = work.tile([C, HID], F32, tag="x_all")
            nc.scalar.dma_start(out=wq_sb, in_=wq_d.ap())
            wqTp = work.tile([128, H, C], F32, tag="sq")
            nc.vector.memset(wqTp, 0.0)
            pwq = pb.tile([128, TCH], F32, tag="b")
            pw4 = pwq.rearrange("p (a c) -> p a c", a=4)
            for hb in range(4):
                nc.tensor.transpose(pw4[:, hb, :],
                                    wq_sb[:, hb * 128:(hb + 1) * 128], ident)
            nc.scalar.copy(out=wqTp[0:D, 0::2, :], in_=pw4[0:D, :, :])
            nc.scalar.copy(out=wqTp[D:128, 1::2, :], in_=pw4[D:128, :, :])
            wqTpr = wides.tile([128, H, C], F32R, tag="wqTpr")
            nc.vector.tensor_copy(out=wqTpr, in_=wqTp)

            def body():
                # ====== ctx path first: it gates kv -> KQ -> sims ======
                cn2 = work.tile([128, 2, CTX], F32, tag="cn2")
                nc.gpsimd.dma_start(
                    out=cn2, in_=ctx_d.ap().rearrange("(g p) c -> p g c", p=128))
                mask8 = small.tile([128, 2], mybir.dt.uint8, tag="m8")
                nc.sync.dma_start(out=mask8,
                                  in_=bass.AP(mask_d, 0, [[1, 128], [128, 2]]))
                x_all = work.tile([128, 32, C], F32, tag="x_all")
                nc.sync.dma_start(
                    out=x_all[:, 0:16, :],
                    in_=xt_d.ap()[0:2048, :].rearrange("(g p) c -> p g c", p=128))
                nc.sync.dma_start(
                    out=x_all[:, 16:32, :],
                    in_=xt_d.ap()[2048:4096, :].rearrange("(g p) c -> p g c", p=128))
                maskf = small.tile([128, 2], F32, tag="mf")
                nc.gpsimd.tensor_copy(out=maskf, in_=mask8)
                # sden[:, kb, j, :]: cols j*64..+64 = mask bcast, rest 0
                sden = iterw.tile([128, 2, 2, 128], F32R, tag="sden")
                for kb in range(2):
                    for j in range(2):
                        nc.gpsimd.tensor_scalar_mul(
                            sden[:, kb, j, D * j:D * j + D], ones64,
                            maskf[:, kb:kb + 1])
                        nc.gpsimd.tensor_scalar_mul(
                            sden[:, kb, j, D * (1 - j):D * (1 - j) + D],
                            ones64, 0.0)
                # ctx LayerNorm (stats DVE, apply Pool)
                csq = work.tile([128, 2, CTX], F32, tag="csq")
                nc.vector.tensor_mul(out=csq, in0=cn2, in1=cn2)
                cst = small.tile([128, 2, 2], F32, tag="cst")
                nc.vector.reduce_sum(out=cst[:, :, 0], in_=cn2,
                                     axis=mybir.AxisListType.X)
                nc.vector.reduce_sum(out=cst[:, :, 1], in_=csq,
                                     axis=mybir.AxisListType.X)
                cmu = small.tile([128, 2, 4], F32, tag="cmu")
                nc.vector.tensor_scalar_mul(cmu[:, :, 0], cst[:, :, 0],
                                            1.0 / CTX)
                nc.vector.tensor_mul(out=cmu[:, :, 1], in0=cmu[:, :, 0],
                                     in1=cmu[:, :, 0])
                nc.vector.scalar_tensor_tensor(out=cmu[:, :, 2],
                                               in0=cst[:, :, 1],
                                               scalar=1.0 / CTX,
                                               in1=cmu[:, :, 1],
                                               op0=MUL, op1=SUB)
                nc.scalar.activation(out=cmu[:, :, 3], in_=cmu[:, :, 2],
                                     func=Sqrt, bias=eps_t)
                crsd = small.tile([128, 2], F32, tag="crsd")
                nc.vector.reciprocal(out=crsd, in_=cmu[:, :, 3])
                cnn = work.tile([128, 2, CTX], F32, tag="csq")
                nc.gpsimd.tensor_tensor(
                    out=cnn, in0=cn2,
                    in1=cmu[:, :, 0].unsqueeze(2).broadcast_to((128, 2, CTX)),
                    op=SUB)
                nc.gpsimd.tensor_tensor(
                    out=cnn, in0=cnn,
                    in1=crsd.unsqueeze(2).broadcast_to((128, 2, CTX)),
                    op=MUL)

                # cn^T via PE transposes; *cw+cb rides the copy
                cnT = wides.tile([128, 6, N], F32R, tag="cnT")
                for rr0 in range(0, 6, 2):
                    pt = pb.tile([128, TCH], F32, tag="b")
                    pt4 = pt.rearrange("p (a c) -> p a c", a=4)
                    for j in range(2):
                        for t in range(2):
                            nc.tensor.transpose(
                                pt4[:, 2 * j + t, :],
                                cnn[:, t, (rr0 + j) * 128:(rr0 + j + 1) * 128],
                                ident)
                    for j in range(2):
                        nc.scalar.activation(
                            out=cnT[:, rr0 + j, :],
                            in_=pt
                            .rearrange("p (a b c) -> p a (b c)", a=2, b=2)[:, j, :],
                            func=Ident, scale=cw_cols[:, rr0 + j:rr0 + j + 1],
                            bias=cb_cols[:, rr0 + j:rr0 + j + 1])
                cnT_r = cnT

                # ========== k^T = Wkv_k^T @ cn^T, minus k_null =========
                # kTr[(h%2)*64+d, h//2, key]
                kTr = wides.tile([128, 4, N], F32R, tag="kTr")
                for half in range(2):
                    pkt = pb.tile([128, TCH], F32, tag="b")
                    pk2 = pkt.rearrange("p (a k) -> p a k", a=2)
                    for blk in range(2):
                        hb = 2 * half + blk
                        for cb in range(6):
                            nc.tensor.matmul(
                                pk2[:, blk, :],
                                wkv_sb[:, cb, hb * 128:(hb + 1) * 128],
                                cnT_r[:, cb, :],
                                start=(cb == 0), stop=(cb == 5))
                    nc.scalar.activation(
                        out=kTr[:, 2 * half:2 * half + 2, :], in_=pk2,
                        func=Ident, bias=nkn_col)
                kTr_r = kTr

                # ===== v -> va = v * mask, zero-padded per head ========
                # va[:, kb, h, :]: cols (h%2)*64..+64 = v_h * mask, rest 0
                va = wides.tile([128, 2, H, 128], F32R, tag="va")
                for kb in range(2):
                    pv = pb.tile([128, TCH], F32, tag="b")
                    for cb in range(6):
                        nc.tensor.matmul(
                            pv,
                            cnT_r[:, cb, kb * 128:(kb + 1) * 128],
                            wkv_sb[:, cb, HID:2 * HID],
                            start=(cb == 0), stop=(cb == 5))
                    pv8 = pv.rearrange("p (h d) -> p h d", h=H)
                    nc.scalar.activation(
                        out=va[:, kb, 0::2, 0:D], in_=pv8[:, 0::2, :],
                        func=Ident, scale=maskf[:, kb:kb + 1])
                    nc.scalar.activation(
                        out=va[:, kb, 1::2, D:128], in_=pv8[:, 1::2, :],
                        func=Ident, scale=maskf[:, kb:kb + 1])
                    opad4 = onespad.rearrange("p (h d) -> p h d", h=4)
                    nc.gpsimd.tensor_scalar_mul(va[:, kb, 0::2, D:128],
                                                opad4, 0.0)
                    nc.gpsimd.tensor_scalar_mul(va[:, kb, 1::2, 0:D],
                                                opad4, 0.0)
                va_r = va

                # ============ KQ_h = Wq_h @ (k_h - k_null)^T ===========
                KQ = wides.tile([128, H, N], F32R, tag="KQ")
                for half in range(4):
                    pkq = pb.tile([128, TCH], F32, tag="b")
                    pq2 = pkq.rearrange("p (a k) -> p a k", a=2)
                    for j in range(2):
                        h = 2 * half + j
                        nc.tensor.matmul(pq2[:, j, :], wqTpr[:, h, :],
                                         kTr_r[:, h // 2, :],
                                         start=True, stop=True)
                    nc.scalar.copy(out=KQ[:, 2 * half:2 * half + 2, :],
                                   in_=pq2)
                KQ_r = KQ

                # ====== xt path: grouped LN -> y -> transposes =========
                # 8 groups of 4 chunks pipeline DVE (stats) -> Pool
                # (normalize) -> PE (transpose) -> Act (copy w/ w,b)
                xnT = iterw.tile([C, TOK], F32R, tag="xnT")
                y = work.tile([128, 32, C], F32, tag="sq")
                xst = small.tile([128, 32, 2], F32, tag="xst")
                xms = small.tile([128, 32, 4], F32, tag="xms")
                for g in range(8):
                    gs = slice(4 * g, 4 * g + 4)
                    xg = x_all[:, gs, :]
                    sqg = work.tile([128, 4, C], F32, tag="sqg")
                    nc.vector.tensor_mul(out=sqg, in0=xg, in1=xg)
                    nc.vector.reduce_sum(out=xst[:, gs, 0], in_=xg,
                                         axis=mybir.AxisListType.X)
                    nc.vector.reduce_sum(out=xst[:, gs, 1], in_=sqg,
                                         axis=mybir.AxisListType.X)
                    nc.vector.tensor_scalar_mul(xms[:, gs, 0],
                                                xst[:, gs, 0], 1.0 / C)
                    nc.vector.tensor_mul(out=xms[:, gs, 1],
                                         in0=xms[:, gs, 0],
                                         in1=xms[:, gs, 0])
                    nc.vector.scalar_tensor_tensor(out=xms[:, gs, 2],
                                                   in0=xst[:, gs, 1],
                                                   scalar=1.0 / C,
                                                   in1=xms[:, gs, 1],
                                                   op0=MUL, op1=SUB)
                    nc.scalar.activation(out=xms[:, gs, 3], in_=xms[:, gs, 2],
                                         func=Sqrt, bias=eps_t)
                    xrsd = small.tile([128, 4], F32, tag="xrsd")
                    nc.vector.reciprocal(out=xrsd, in_=xms[:, gs, 3])
                    yg = y[:, gs, :]
                    nc.gpsimd.tensor_tensor(
                        out=yg, in0=xg,
                        in1=xms[:, gs, 0].unsqueeze(2)
                        .broadcast_to((128, 4, C)), op=SUB)
                    nc.gpsimd.tensor_tensor(
                        out=yg, in0=yg,
                        in1=xrsd.unsqueeze(2).broadcast_to((128, 4, C)),
                        op=MUL)
                    pt = pb.tile([128, TCH], F32, tag="b")
                    pt4 = pt.rearrange("p (a c) -> p a c", a=4)
                    for j in range(4):
                        nc.tensor.transpose(pt4[:, j, :], yg[:, j, :], ident)
                    nc.scalar.activation(
                        out=xnT[:, g * TCH:(g + 1) * TCH], in_=pt,
                        func=Ident, scale=nw_col, bias=nb_col)
                xnT_r = xnT

                # ===== attention: software-pipelined (pair,chunk) units =====
                def flush_unit(u):
                    ut, up, upes, uo = u
                    utsl = slice(ut * TCH, (ut + 1) * TCH)
                    po = pb.tile([128, TCH], F32, tag="b")
                    for j in range(2):
                        for kb in range(2):
                            nc.tensor.matmul(
                                po, va[:, kb, 2 * up + j, :],
                                upes[j][:, kb, :],
                                start=(j == 0 and kb == 0),
                                stop=(j == 1 and kb == 1),
                                skip_group_check=True)
                    pden = pb.tile([128, TCH], F32, tag="b")
                    nc.tensor.matmul(pden, onescol, ones_row,
                                     start=True, stop=False,
                                     skip_group_check=True)
                    for j in range(2):
                        for kb in range(2):
                            nc.tensor.matmul(
                                pden, sden[:, kb, j, :], upes[j][:, kb, :],
                                start=False, stop=(j == 1 and kb == 1),
                                skip_group_check=True)
                    rden = rp.tile([128, TCH], F32, tag="r")
                    nc.vector.reciprocal_approx_fast(out=rden, in_=pden)
                    nc.vector.scalar_tensor_tensor(
                        out=uo[:, up, :], in0=po, scalar=vn2_col, in1=rden,
                        op0=ADD, op1=MUL)
                    if up == NP - 1:
                        # out-proj + residual seed + bias for chunk ut
                        pf = pb.tile([128, TCH], F32, tag="b")
                        nc.tensor.matmul(pf, ident_r, xnT_r[:, utsl],
                                         start=True, stop=False,
                                         skip_group_check=True)
                        for pp in range(NP):
                            nc.tensor.matmul(pf, wout_pr[:, pp, :],
                                             uo[:, pp, :],
                                             start=False, stop=(pp == NP - 1),
                                             skip_group_check=True)
                        fTc = ftp.tile([C, TCH], F32, tag="ft")
                        nc.vector.tensor_scalar_add(fTc, pf, bout_col)
                        # output stays [C, TOK] in DRAM; the host transposes
                        nc.sync.dma_start(out=out_d.ap()[:, utsl], in_=fTc)

                pend = None
                for t in range(NT):
                    tsl = slice(t * TCH, (t + 1) * TCH)
                    o_chunk = op_pool.tile([128, NP, TCH], F32R, tag="o")
                    for p in range(NP):
                        pes = []
                        for j in range(2):
                            h = 2 * p + j
                            psim = pa.tile([128, 2, TCH], F32, tag="a")
                            for kb in range(2):
                                nc.tensor.matmul(
                                    psim[:, kb, :],
                                    KQ_r[:, h, kb * 128:(kb + 1) * 128],
                                    xnT_r[:, tsl], start=True, stop=True)
                            pe = pep.tile([128, 2, TCH], F32R, tag="pe")
                            nc.scalar.activation(out=pe, in_=psim, func=Exp,
                                                 scale=SCALE)
                            pes.append(pe)
                        u = (t, p, pes, o_chunk)
                        if pend is not None:
                            flush_unit(pend)
                        pend = u
                flush_unit(pend)

            if n_iters >= 1:
                with tc.For_i(0, n_iters, 1):
                    body()

    nc.compile()
    return nc


_CACHE = {}


def get_nc(n_iters: int = 1):
    if n_iters not in _CACHE:
        _CACHE[n_iters] = build(n_iters)
    return _CACHE[n_iters]


def make_in_maps(xt, context, mask, norm_w, norm_b, ctx_norm_w, ctx_norm_b,
                 Wq, Wkv, null_kv, Wout, bout):
    xt = np.asarray(xt, dtype=np.float32).reshape(B, TOK, C)
    context = np.asarray(context, dtype=np.float32)
    mask8 = np.asarray(mask).astype(np.uint8)
    shared = {
        "norm_w": np.asarray(norm_w, np.float32),
        "norm_b": np.asarray(norm_b, np.float32),
        "ctx_norm_w": np.asarray(ctx_norm_w, np.float32),
        "ctx_norm_b": np.asarray(ctx_norm_b, np.float32),
        "Wq": np.asarray(Wq, np.float32),
        "Wkv": np.asarray(Wkv, np.float32),
        "null_kv": np.asarray(null_kv, np.float32),
        "Wout": np.asarray(Wout, np.float32),
        "bout": np.asarray(bout, np.float32),
    }
    return [
        {"xt": xt[b], "context": context[b], "mask": mask8[b], **shared}
        for b in range(B)
    ]


def kernel(xt, context, mask, norm_w, norm_b, ctx_norm_w, ctx_norm_b,
           Wq, Wkv, null_kv, Wout, bout):
    nc = get_nc(1)
    in_maps = make_in_maps(xt, context, mask, norm_w, norm_b, ctx_norm_w,
                           ctx_norm_b, Wq, Wkv, null_kv, Wout, bout)
    res = run_bass_kernel_spmd(nc, in_maps, core_ids=list(range(NCORES)))
    out = np.stack([res.results[b]["out"] for b in range(B)], axis=0)
    # device layout is [C, TOK]; untranspose on the host
    return (out.transpose(0, 2, 1).reshape(B, XS, YS, C)
            .astype(np.float32, copy=False))
